# revision 52
# baseline (speedup 1.0000x reference)
"""AxialAttention TRN2 kernel, v5.

Sharding: 8 cores = 4 batches x 2 head-groups (4 heads each), all-bf16 compute.

v5 over the original baseline (reps-differenced HW: 830-893us -> ~490us):
  - bias matmuls removed from PE entirely: k-bias cancels in softmax
    (constant along the attended axis), v-bias folds into the host-side
    output bias (softmax rows sum to 1 => out += 2*Wout@bv exactly), and
    only the q-bias is applied on device (DVE tensor_tensor at p1 drain
    against a precomputed broadcast tile).
  - the qkR DRAM round trip (17MB write + 17MB read per core) is gone:
    row-block ch-major q,k tiles are produced by PE transposes (identity
    matmuls, bf16 PSUM out) fused per-pt into p1, drained to SBUF on
    Act/DVE. The kernel was HBM-contention-bound across 8 cores (4-core
    body time 488us vs 643us on 8), so this traffic cut bought far more
    on HW than the ~30us of added PE time.
  - O accumulator split into 4 parity classes by col-chunk%4 (tiles
    O_par[pair][c4], cols ordered (chunk//4, w, h)) so whole-tile dep
    tracking never false-serializes col-block Pool adds (chunk j),
    previous-chunk adds (j-1), and p3 reads (j-2).
  - latency-critical DMAs on HWDGE queues (vL loads and p3 out writes on
    Act) since big scatter DMAs (1024 descriptors) monopolize the 1024-entry
    SWDGE ring and convoy the Pool queue; only x prefetch and the qkvC
    write (huge slack) stay on Pool.
  - col block PSUM fully double-buffered: psO as [128,512] halves bufs=3,
    psF [128,512] bufs=2, psZ [128,512] bufs=1; row: psQK 2, psT 1.

Per core, two software-pipelined blocks:

ROW block (16 chunks of 8 rows, p1 + PE-transposes fused with row
attention at lag 3):
  - qkv projection s-major (x-stationary matmuls): psum [px, ch] drains to
    an SBUF s-major chunk tile (q+bq on DVE, k on Act, v on DVE).
  - per pt, 4 PE transposes [128 px, 128 ch] -> bf16 psT -> one strided
    drain into the ch-major qkt tile [128, 4cht, 1024px] (pixels (h, w)).
  - the chunk tile is also written to DRAM once, col-order: qkvC [S, 768]
    (rows (w, h)), feeding the whole col block.
  - row attention per chunk: scores (contract d=64 on partitions) -> exp
    (Act, 512-wide halves) -> pair-stacked Z (partition-offset ones-
    matmuls) -> f32 reciprocal per half (DVE) -> PV with UNNORMALIZED P ->
    drain = tensor_tensor MULT(psO, rz) scattered into the 4 O_par parity
    tiles (DVE), so normalization is free.

COL block (16 chunks of 8 cols + output projection, p3 at lag 2):
  - q,k via XBAR transpose DMAs of contiguous qkvC rows (SP queue, which
    carries ONLY transposes - XBAR transpose<->copy interleave corrupts
    data); v via a plain strided load on Act.
  - PV in [128,512] halves; drain: psO*rz on DVE into a temp, then
    SBUF-only add into O_par[pair][j%4] on Pool (GPSIMD cannot touch PSUM).
  - p3 output projection per finished col chunk, bf16 out in col-major
    pixel order; bout+2*Wout@bv and the final (W,H)->(H,W) transpose happen
    on host.

Engine/queue notes: DMAs occupy the issuing queue; only SP and Act have
HWDGE, Pool is SWDGE. All PSUM-reading drains sit on Act/DVE. PE does
matmuls + transposes only.
"""

import numpy as np
import ml_dtypes
from contextlib import ExitStack

import concourse.bass as bass
import concourse.bacc as bacc
import concourse.tile as tile
from concourse import mybir
from concourse.bass_utils import run_bass_kernel_spmd

C = 512          # channels
H = 128
W = 128
S = H * W        # 16384 pixels
NHC = 4          # heads per core
D = 64           # head dim
GC = NHC * D     # 256 q (or k, or v) channels per core
QK = 2 * GC      # 512 q+k channels
QKV = 3 * GC     # 768 qkv channels
CT = C // 128    # 4 contraction tiles
NCH = 16         # chunks (8 rows or 8 cols each)
SCALE = 1.0 / np.sqrt(D)

F32 = mybir.dt.float32
BF16 = mybir.dt.bfloat16
EXP = mybir.ActivationFunctionType.Exp
ADD = mybir.AluOpType.add
MULT = mybir.AluOpType.mult

_CACHED_NC = None


def _mix(primary, secondary):
    """Primary unit list with secondary units spread evenly between."""
    np_, ns = len(primary), len(secondary)
    if np_ == 0:
        return list(secondary)
    out = []
    emitted = 0
    for i, u in enumerate(primary):
        out.append(u)
        want = (i + 1) * ns // np_
        while emitted < want:
            out.append(secondary[emitted])
            emitted += 1
    out.extend(secondary[emitted:])
    return out


def _interleave(primary, secondary):
    for u in _mix(primary, secondary):
        u()


def build_nc(reps=1):
    nc = bacc.Bacc()
    x_in = nc.dram_tensor("x", [C, S], BF16, kind="ExternalInput")
    wqkvT = nc.dram_tensor("wqkvT", [C, QKV], BF16, kind="ExternalInput")
    bq_in = nc.dram_tensor("bq", [1, GC], BF16, kind="ExternalInput")
    eye_in = nc.dram_tensor("eye", [128, 128], BF16, kind="ExternalInput")
    woutT = nc.dram_tensor("woutT", [GC, C], BF16, kind="ExternalInput")
    out = nc.dram_tensor("out", [C, S], BF16, kind="ExternalOutput")

    with tile.TileContext(nc) as tc, ExitStack() as ctx:
        persist = ctx.enter_context(tc.tile_pool(name="persist", bufs=1))
        dram = ctx.enter_context(tc.tile_pool(name="dram", bufs=1, space="DRAM"))

        wqkv_sb = persist.tile([128, CT, QKV], BF16, tag="wqkv")
        nc.scalar.dma_start(
            out=wqkv_sb, in_=wqkvT.ap().rearrange("(t p) o -> p t o", p=128)
        )
        bq_sb = persist.tile([1, GC], BF16, tag="bq")
        nc.scalar.dma_start(out=bq_sb, in_=bq_in.ap())
        wout_sb = persist.tile([128, 2, C], BF16, tag="wout")
        nc.scalar.dma_start(
            out=wout_sb, in_=woutT.ap().rearrange("(t p) o -> p t o", p=128)
        )
        eye_sb = persist.tile([128, 128], BF16, tag="eye")
        nc.scalar.dma_start(out=eye_sb, in_=eye_in.ap())
        ones1 = persist.tile([1, 128], BF16, tag="ones1")
        nc.vector.memset(ones1, 1.0)
        onesZ = persist.tile([128, 64], BF16, tag="onesZ")
        nc.vector.memset(onesZ, 1.0)
        # broadcast q-bias across 128 partitions via a K=1 ones-matmul (once)
        bq_bc = persist.tile([128, GC], BF16, tag="bq_bc")
        with tc.tile_pool(name="init_ps", bufs=1, space="PSUM") as initp:
            psb = initp.tile([128, GC], F32)
            nc.tensor.matmul(out=psb, lhsT=ones1, rhs=bq_sb, start=True, stop=True)
            nc.scalar.copy(out=bq_bc, in_=psb)
        # O accumulator: 2 head-pairs x 4 parity classes, cols (chunk//4, w, h)
        O_par = [
            [
                persist.tile([128, 4 * 1024], BF16, tag=f"O{p}c{c}", name=f"O{p}c{c}")
                for c in range(4)
            ]
            for p in range(2)
        ]

        qkvC = dram.tile([S, QKV], BF16, tag="qkvC")  # qkv s-major col-order

        for _ in range(reps):
            build_body(nc, tc, x_in, wqkv_sb, bq_bc, wout_sb,
                       onesZ, eye_sb, O_par, qkvC, out)

    nc.finalize()
    return nc


def build_body(nc, tc, x_in, wqkv_sb, bq_bc, wout_sb, onesZ, eye_sb,
               O_par, qkvC, out):
    x_r = x_in.ap().rearrange("(t p) s -> p t s", p=128)
    # col-order row-chunk write view: partitions = w, then (h, ch)
    qkvC_w = qkvC[:].rearrange("(w h) d -> w h d", h=H)

    def make_attn_units(qk_t, v_t, voff, pchpool, rzpool, psSp, psZp, psOp,
                        drain_half, s_halves, o_halves):
        """One chunk of axial attention (4 heads as 2 pairs).
        qk_t: [128, 4, 1024] ch-major (cht 0,1 = q; 2,3 = k).
        v_t: [128, 8, *] s-major, head h at cols voff+h*64.
        s_halves: emit scores/exp at [*,512] half-granularity.
        o_halves: PV + drain at [*,512] half-granularity (col block).
        drain_half(p, half, psO_ap, rz_ap): normalize+store one PV result."""
        pchs = {}
        rzs = {}
        units = []

        def s_unit(h):
            def emit():
                r0 = (h % 2) * 64
                cq = h // 2
                pch = pchpool.tile([128, 1024], BF16, name="pch")
                pchs[h] = pch
                nh = 2 if s_halves else 1
                for j in range(nh):
                    w_ = 1024 // nh
                    psS = psSp.tile([128, w_], F32, name="psS")
                    for ii in range(w_ // 128):
                        i = (w_ // 128) * j + ii
                        nc.tensor.matmul(
                            out=psS[:, ii * 128 : (ii + 1) * 128],
                            lhsT=qk_t[r0 : r0 + 64, 2 + cq, i * 128 : (i + 1) * 128],
                            rhs=qk_t[r0 : r0 + 64, cq, i * 128 : (i + 1) * 128],
                            start=True, stop=True,
                        )
                    nc.scalar.activation(
                        out=pch[:, j * w_ : (j + 1) * w_], in_=psS,
                        func=EXP, scale=float(SCALE),
                    )
            return emit

        def z_unit(p):
            def emit():
                rz = rzpool.tile([128, 1024], F32, name="rz")
                rzs[p] = rz
                for j in range(2):
                    psZ = psZp.tile([128, 512], F32, name="psZ")
                    for hl in range(2):
                        r0 = hl * 64
                        nc.tensor.matmul(
                            out=psZ[r0 : r0 + 64, :],
                            lhsT=onesZ,
                            rhs=pchs[2 * p + hl][:, j * 512 : (j + 1) * 512],
                            start=True, stop=True,
                        )
                    nc.vector.reciprocal_approx_fast(
                        out=rz[:, j * 512 : (j + 1) * 512], in_=psZ
                    )
            return emit

        def pv_unit(p):
            def emit():
                nhalf = 2 if o_halves else 1
                for half in range(nhalf):
                    wd = 1024 // nhalf
                    psO_t = psOp.tile([128, wd], F32, name="psO")
                    for hl in range(2):
                        h = 2 * p + hl
                        r0 = hl * 64
                        for i in range(wd // 128):
                            iw = (wd // 128) * half + i
                            nc.tensor.matmul(
                                out=psO_t[r0 : r0 + 64, i * 128 : (i + 1) * 128],
                                lhsT=v_t[:, iw, voff + h * 64 : voff + (h + 1) * 64],
                                rhs=pchs[h][:, iw * 128 : (iw + 1) * 128],
                                start=True, stop=True,
                            )
                    drain_half(p, half if o_halves else None, psO_t,
                               rzs[p][:, half * wd : (half + 1) * wd])
            return emit

        units.append(s_unit(0))
        units.append(s_unit(1))
        units.append(z_unit(0))
        units.append(s_unit(2))
        units.append(pv_unit(0))
        units.append(s_unit(3))
        units.append(z_unit(1))
        units.append(pv_unit(1))
        return units

    # ---------------- row block: p1 + row attention, lag-2 pipeline ----------
    with (
        tc.tile_pool(name="r_x", bufs=5) as xpool,
        tc.tile_pool(name="r_qkv", bufs=4) as qkvpool,
        tc.tile_pool(name="r_qkt", bufs=3) as qktpool,
        tc.tile_pool(name="r_pch", bufs=3) as pchpool,
        tc.tile_pool(name="r_rz", bufs=2) as rzpool,
        tc.tile_pool(name="r_psqk", bufs=2, space="PSUM") as psQKp,
        tc.tile_pool(name="r_psv", bufs=1, space="PSUM") as psVp,
        tc.tile_pool(name="r_pss", bufs=1, space="PSUM") as psSp,
        tc.tile_pool(name="r_psz", bufs=1, space="PSUM") as psZp,
        tc.tile_pool(name="r_pso", bufs=1, space="PSUM") as psOp,
        tc.tile_pool(name="r_pst", bufs=1, space="PSUM") as psTp,
    ):
        xgs = {}
        xgs[0] = xpool.tile([128, CT, 1024], BF16, name="xg")
        nc.gpsimd.dma_start(out=xgs[0], in_=x_r[:, :, 0:1024])
        qkvs = {}   # n -> s-major chunk tile
        qkts = {}   # n -> transposed ch-major q,k tile

        def make_p1_units(n):
            qkv = qkvpool.tile([128, 8, QKV], BF16, name="qkv")
            qkvs[n] = qkv
            qkt = qktpool.tile([128, CT, 1024], BF16, name="qkt")
            qkts[n] = qkt
            units = []

            def prefetch():
                if n + 1 < NCH:
                    xg = xpool.tile([128, CT, 1024], BF16, name="xg")
                    xgs[n + 1] = xg
                    nc.gpsimd.dma_start(
                        out=xg, in_=x_r[:, :, (n + 1) * 1024 : (n + 2) * 1024]
                    )
            units.append(prefetch)

            def qk_unit(pt):
                def emit():
                    ps = psQKp.tile([128, QK], F32)
                    for ct in range(CT):
                        nc.tensor.matmul(
                            out=ps,
                            lhsT=xgs[n][:, ct, pt * 128 : (pt + 1) * 128],
                            rhs=wqkv_sb[:, ct, 0:QK],
                            start=(ct == 0), stop=(ct == CT - 1),
                        )
                    # q gets its bias here; k needs none (cancels in softmax)
                    nc.vector.tensor_tensor(
                        out=qkv[:, pt, 0:GC], in0=ps[:, 0:GC], in1=bq_bc, op=ADD
                    )
                    nc.scalar.copy(out=qkv[:, pt, GC:QK], in_=ps[:, GC:QK])
                return emit

            def v_unit(pt):
                def emit():
                    ps = psVp.tile([128, GC], F32)
                    for ct in range(CT):
                        nc.tensor.matmul(
                            out=ps,
                            lhsT=xgs[n][:, ct, pt * 128 : (pt + 1) * 128],
                            rhs=wqkv_sb[:, ct, QK:QKV],
                            start=(ct == 0), stop=(ct == CT - 1),
                        )
                    # v bias is folded into the host-side output bias
                    nc.vector.tensor_copy(out=qkv[:, pt, QK:QKV], in_=ps)
                return emit

            def t_unit(pt):
                # on-chip q,k transpose: [128 px, 128 ch] -> bf16 PSUM ->
                # ch-major qkt slice; replaces the qkR DRAM round trip
                def emit():
                    psT = psTp.tile([128, CT, 128], BF16, name="psT")
                    for cht in range(CT):
                        nc.tensor.transpose(
                            out=psT[:, cht, :],
                            in_=qkv[:, pt, cht * 128 : (cht + 1) * 128],
                            identity=eye_sb,
                        )
                    dst = qkt[:, :, pt * 128 : (pt + 1) * 128]
                    if pt % 2 == 0:
                        nc.scalar.copy(out=dst, in_=psT)
                    else:
                        nc.vector.tensor_copy(out=dst, in_=psT)
                return emit

            h0 = n * 8
            def w_col():
                nc.gpsimd.dma_start(
                    out=qkvC_w[:, h0 : h0 + 8, :], in_=qkv,
                )

            for pt in range(8):
                units.append(qk_unit(pt))
                units.append(v_unit(pt))
                units.append(t_unit(pt))
            units.append(w_col)
            return units

        def row_drain(m):
            h0 = m * 8
            def drain(p, half, psO_t, rz_t):
                # psO cols (h=8, w=128); w = 32*ci + 8*c4 + wi
                po = psO_t[:, :].rearrange(
                    "q (h ci c4 w) -> q c4 ci w h", h=8, ci=4, c4=4, w=8
                )
                rzv = rz_t[:, :].rearrange(
                    "q (h ci c4 w) -> q c4 ci w h", h=8, ci=4, c4=4, w=8
                )
                for c4 in range(4):
                    nc.vector.tensor_tensor(
                        out=O_par[p][c4][:, :].rearrange(
                            "q (ci w h) -> q ci w h", ci=4, w=8, h=H
                        )[:, :, :, h0 : h0 + 8],
                        in0=po[:, c4],
                        in1=rzv[:, c4],
                        op=MULT,
                    )
            return drain

        for n in range(NCH + 3):
            p1 = make_p1_units(n) if n < NCH else []
            if n >= 3:
                m = n - 3
                at = make_attn_units(
                    qkts[m], qkvs[m], QK, pchpool, rzpool, psSp, psZp, psOp,
                    row_drain(m), s_halves=True, o_halves=False,
                )
            else:
                at = []
            for u in (_mix(p1, at) if p1 else list(at)):
                u()

    # ---------------- col block: col attention + p3, lag-2 pipeline ----------
    out_r = out.ap().rearrange("(t p) s -> p t s", p=128)
    with (
        tc.tile_pool(name="c_qkt", bufs=5) as qktcpool,
        tc.tile_pool(name="c_v", bufs=6) as vlpool,
        tc.tile_pool(name="c_pch", bufs=6) as pchpool,
        tc.tile_pool(name="c_rz", bufs=4) as rzpool,
        tc.tile_pool(name="c_oc", bufs=6) as ocpool,
        tc.tile_pool(name="c_out", bufs=4) as outpool,
        tc.tile_pool(name="c_pss", bufs=2, space="PSUM") as psSp,
        tc.tile_pool(name="c_psz", bufs=1, space="PSUM") as psZp,
        tc.tile_pool(name="c_pso", bufs=3, space="PSUM") as psOp,
        tc.tile_pool(name="c_psf", bufs=2, space="PSUM") as psFp,
    ):
        loads = {}

        def load_chunk(j):
            vL = vlpool.tile([128, 8, GC], BF16, name="vL")
            nc.scalar.dma_start(
                out=vL,
                in_=qkvC[j * 1024 : (j + 1) * 1024, QK:QKV].rearrange(
                    "(t p) d -> p t d", p=128
                ),
            )
            qkt = qktcpool.tile([128, CT, 1024], BF16, name="qktc")
            for cht in range(CT):
                nc.sync.dma_start_transpose(
                    out=qkt[:, cht, :],
                    in_=qkvC[j * 1024 : (j + 1) * 1024,
                             cht * 128 : (cht + 1) * 128],
                )
            loads[j] = (qkt, vL)

        load_chunk(0)
        load_chunk(1)

        def col_drain(j):
            c4, ci = j % 4, j // 4
            def drain(p, half, psO_t, rz_t):
                oc = ocpool.tile([128, 512], BF16, name="oc")
                nc.vector.tensor_tensor(out=oc, in0=psO_t, in1=rz_t, op=MULT)
                dst = O_par[p][c4][:, ci * 1024 + half * 512 :
                                   ci * 1024 + (half + 1) * 512]
                nc.gpsimd.tensor_tensor(out=dst, in0=dst, in1=oc, op=ADD)
            return drain

        def make_p3_units(m):
            c4, ci = m % 4, m // 4
            units = []
            outsbs = {}

            def f_unit(pg, ot):
                def emit():
                    if ot == 0:
                        outsbs[pg] = outpool.tile(
                            [128, CT, 512], BF16, name="outsb"
                        )
                    psf = psFp.tile([128, 512], F32)
                    off = ci * 1024 + pg * 512
                    for ic in range(2):
                        nc.tensor.matmul(
                            out=psf,
                            lhsT=wout_sb[:, ic, ot * 128 : (ot + 1) * 128],
                            rhs=O_par[ic][c4][:, off : off + 512],
                            start=(ic == 0), stop=(ic == 1),
                        )
                    # bias (bout + 2*Wout@bv) is added on the host
                    if ot % 2 == 0:
                        nc.scalar.copy(out=outsbs[pg][:, ot, :], in_=psf)
                    else:
                        nc.vector.tensor_copy(out=outsbs[pg][:, ot, :], in_=psf)
                    if ot == 3:
                        goff = m * 1024 + pg * 512
                        nc.scalar.dma_start(
                            out=out_r[:, :, goff : goff + 512], in_=outsbs[pg]
                        )
                return emit

            for pg in range(2):
                for ot in range(4):
                    units.append(f_unit(pg, ot))
            return units

        for j in range(NCH + 2):
            at = []
            if j < NCH:
                def prefetch(jj=j):
                    if jj + 2 < NCH:
                        load_chunk(jj + 2)
                qkt, vL = loads[j]
                at = make_attn_units(
                    qkt, vL, 0, pchpool, rzpool, psSp, psZp, psOp,
                    col_drain(j), s_halves=True, o_halves=True,
                )
                # prefetch after S0 so Pool's O_par adds aren't queued
                # behind next chunk's vL load
                at = at[:1] + [prefetch] + at[1:]
            p3 = make_p3_units(j - 2) if j >= 2 else []
            if at:
                _interleave(at, p3)
            else:
                _interleave(p3, [])


def get_nc():
    global _CACHED_NC
    if _CACHED_NC is None:
        _CACHED_NC = build_nc()
    return _CACHED_NC


def make_in_maps(x, Wqkv, bqkv, Wout, bout):
    """Per-core input dicts: core c = (b, g) with b = c // 2, g = c % 2."""
    bf16 = ml_dtypes.bfloat16
    in_maps = []
    for c in range(8):
        b, g = c // 2, c % 2
        sel = slice(256 * g, 256 * (g + 1))
        wq = Wqkv[sel, :]
        wk = Wqkv[512 + 256 * g : 512 + 256 * (g + 1), :]
        wv = Wqkv[1024 + 256 * g : 1024 + 256 * (g + 1), :]
        bq = bqkv[sel]
        w_all = np.concatenate([wq, wk, wv], axis=0)      # [768, 512]
        in_maps.append(
            {
                "x": np.ascontiguousarray(x[b].reshape(C, S)).astype(bf16),
                "wqkvT": np.ascontiguousarray(w_all.T).astype(bf16),
                "bq": bq.reshape(1, GC).astype(bf16),
                "woutT": np.ascontiguousarray(Wout[:, sel].T).astype(bf16),
                "eye": np.eye(128, dtype=bf16),
            }
        )
    return in_maps


def assemble_output(results, B, bout, Wout, bqkv):
    """results: list of 8 per-core dicts with 'out' [C, S] bf16 in col-major
    pixel order (no bias). Returns [B, C, H, W] float32 with the full bias
    (bout + 2*Wout@bv, the folded v-bias) added."""
    bv = np.asarray(bqkv, np.float32)[2 * C : 3 * C]
    bias = np.asarray(bout, np.float32) + 2.0 * (
        np.asarray(Wout, np.float32) @ bv
    )
    out = np.empty((B, C, H, W), dtype=np.float32)
    for b in range(B):
        acc = (
            results[2 * b]["out"].astype(np.float32)
            + results[2 * b + 1]["out"].astype(np.float32)
        )
        out[b] = acc.reshape(C, W, H).swapaxes(1, 2)
    out += bias[None, :, None, None]
    return out


def kernel(x, Wqkv, bqkv, Wout, bout):
    x = np.asarray(x, dtype=np.float32)
    Wqkv = np.asarray(Wqkv, dtype=np.float32)
    bqkv = np.asarray(bqkv, dtype=np.float32)
    Wout = np.asarray(Wout, dtype=np.float32)
    bout = np.asarray(bout, dtype=np.float32)

    nc = get_nc()
    in_maps = make_in_maps(x, Wqkv, bqkv, Wout, bout)
    res = run_bass_kernel_spmd(nc, in_maps, core_ids=list(range(8)))
    return assemble_output(res.results, x.shape[0], bout, Wout, bqkv)
